# revision 12
# baseline (speedup 1.0000x reference)
"""Trainium2 Bass kernel for nn_CLloss (contrastive loss, anchor row 0).

Math (faithful to the torch/jax reference):
    e_j = x_j / max(||x_j||, 1e-12)          (row-normalize embed)
    d_j = ||(e_0 + 1e-6) - e_j||_2           (pairwise distance to anchor, j>=1)
    log_sim_j = -d_j / 0.1
    c_j = <labels_j, labels_0>
    Ci = 1e-12 + sum c_j ; Ei = 1e-12 + sum exp(log_sim_j)
    Li = sum -(c_j/Ci) * (log_sim_j - log Ei) ; loss = Li / n

With a = e_0 + 1e-6:  d_j^2 = ||a||^2 + 1 - 2*(a . x_j)/||x_j||, so the only
O(n*d) work is two per-row contractions over the feature dim: a.x_j and
sum_k x_jk^2.  We shard rows across 8 cores.  Each core receives its shard
TRANSPOSED ([feature k, row j], done on host) so the feature dim lies on SBUF
partitions; the tensor engine then contracts over partitions:
  - a.x   via matmul(lhsT=[a_chunk | ones], rhs=x_tile)
  - sum x^2 via matmul(same lhsT, rhs=square(x_tile)), square on the scalar
    engine (its free-dim rate keeps it under the DMA roofline).
Both accumulate across the 16 feature chunks in PSUM.  Device returns per-row
(a.x, sum x^2); host does the O(n) epilogue in float64.
"""

import numpy as np

import concourse.bacc as bacc
import concourse.bass as bass
import concourse.tile as tile
from concourse import mybir
from concourse.bass_utils import run_bass_kernel_spmd

N_ROWS = 16384
DIM = 2048
N_CORES = 8
ROWS_PER_CORE = N_ROWS // N_CORES  # 2048
KC = DIM // 128  # 16 feature chunks of 128 partitions
JC = ROWS_PER_CORE // 512  # 4 row chunks of 512 (fp32 matmul max free dim)

PD_EPS = 1e-6
NORM_EPS = 1e-12
T = 0.1

_NC_CACHE = {}


def _build_bass():
    # Bacc (not raw Bass): its compile() legalizes sync waits — walrus accepts
    # at most ONE wait per instruction, and Tile freely emits several.
    nc = bacc.Bacc()
    f32 = mybir.dt.float32
    xt = nc.dram_tensor("xt", [DIM, ROWS_PER_CORE], f32, kind="ExternalInput")
    # Per feature chunk c, 4 weight columns: [a_c, 0, 0, 1].  The x-matmul
    # uses cols (a_c, 0) and the x^2-matmul cols (0, 1), so both can
    # accumulate into the SAME psum tile: row 0 collects a.x only, row 1
    # collects sum x^2 only.
    aw = nc.dram_tensor("aw", [128, 4 * KC], f32, kind="ExternalInput")
    out = nc.dram_tensor("out", [2, ROWS_PER_CORE], f32, kind="ExternalOutput")

    with tile.TileContext(nc) as tc:
        with (
            tc.tile_pool(name="xp", bufs=3) as xp,
            tc.tile_pool(name="sqp", bufs=3) as sqp,
            tc.tile_pool(name="singles", bufs=1) as singles,
            tc.tile_pool(name="psum", bufs=1, space="PSUM") as psum,
        ):
            aw_sb = singles.tile([128, 4 * KC], f32)
            nc.sync.dma_start(out=aw_sb[:], in_=aw[:])

            # Warmup matmul reading only aw_sb: makes PE observe the aw DMA
            # semaphore here, so the first real matmul carries a single sync
            # wait (the S3 ldweights slot only fits one).
            ps_warm = psum.tile([2, 2], f32, tag="pswarm", name="pswarm")
            nc.tensor.matmul(
                ps_warm[:], aw_sb[:, 0:2], aw_sb[:, 0:2], start=True, stop=True
            )

            ps = [
                psum.tile([2, 512], f32, tag=f"ps{j}", name=f"ps{j}")
                for j in range(JC)
            ]

            for c in range(KC):
                x_tile = xp.tile([128, ROWS_PER_CORE], f32)
                nc.sync.dma_start(
                    out=x_tile[:], in_=xt[c * 128 : (c + 1) * 128, :]
                )
                sq_tile = sqp.tile([128, ROWS_PER_CORE], f32)
                nc.scalar.activation(
                    out=sq_tile[:],
                    in_=x_tile[:],
                    func=mybir.ActivationFunctionType.Square,
                )
                w_x = aw_sb[:, 4 * c : 4 * c + 2]  # [a_c | 0]
                w_q = aw_sb[:, 4 * c + 2 : 4 * c + 4]  # [0 | 1]
                for j in range(JC):
                    nc.tensor.matmul(
                        ps[j][:],
                        w_x,
                        x_tile[:, j * 512 : (j + 1) * 512],
                        start=(c == 0),
                        stop=False,
                    )
                for j in range(JC):
                    nc.tensor.matmul(
                        ps[j][:],
                        w_q,
                        sq_tile[:, j * 512 : (j + 1) * 512],
                        start=False,
                        stop=(c == KC - 1),
                    )

            out_sb = singles.tile([2, ROWS_PER_CORE], f32)
            for j in range(JC):
                nc.vector.tensor_copy(
                    out_sb[0:2, j * 512 : (j + 1) * 512], ps[j][:]
                )
            nc.sync.dma_start(out=out[:], in_=out_sb[:])

    nc.compile()
    return nc


def _get_nc():
    if "nc" not in _NC_CACHE:
        _NC_CACHE["nc"] = _build_bass()
    return _NC_CACHE["nc"]


def _make_in_maps(embed):
    x0 = embed[0].astype(np.float64)
    nrm0 = max(np.sqrt(np.dot(x0, x0)), NORM_EPS)
    a64 = x0 / nrm0 + PD_EPS
    a32 = a64.astype(np.float32)

    aw = np.zeros((128, 4 * KC), np.float32)
    for c in range(KC):
        aw[:, 4 * c] = a32[c * 128 : (c + 1) * 128]
        aw[:, 4 * c + 3] = 1.0

    in_maps = []
    for core in range(N_CORES):
        shard = embed[core * ROWS_PER_CORE : (core + 1) * ROWS_PER_CORE]
        xt = np.ascontiguousarray(shard.T)  # [DIM, ROWS_PER_CORE]
        in_maps.append({"xt": xt, "aw": aw})
    return in_maps, a64


def _epilogue(results, a64, labels):
    adot = np.concatenate([r["out"][0] for r in results]).astype(np.float64)
    ss = np.concatenate([r["out"][1] for r in results]).astype(np.float64)

    nrm = np.maximum(np.sqrt(ss), NORM_EPS)
    t = adot / nrm  # a . e_j
    a2 = np.dot(a64, a64)
    d2 = np.maximum(a2 + 1.0 - 2.0 * t, 0.0)
    d = np.sqrt(d2)[1:]  # anchor row excluded, j = 1..n-1

    lab = labels.astype(np.float64)
    c = lab[1:] @ lab[0]
    ci = 1e-12 + c.sum()
    log_sim = -d / T
    ei = 1e-12 + np.exp(log_sim).sum()
    li = (-(c / ci) * (log_sim - np.log(ei))).sum()
    return np.asarray(li / N_ROWS, dtype=np.float32)


def _run(embed, labels, trace=False):
    embed = np.ascontiguousarray(np.asarray(embed, dtype=np.float32))
    labels = np.asarray(labels)
    assert embed.shape == (N_ROWS, DIM), embed.shape

    nc = _get_nc()
    in_maps, a64 = _make_in_maps(embed)
    kwargs = {"trace_cores": list(range(N_CORES))} if trace else {}
    res = run_bass_kernel_spmd(
        nc, in_maps, core_ids=list(range(N_CORES)), trace=trace, **kwargs
    )
    return _epilogue(res.results, a64, labels), res


def kernel(embed, labels):
    out, _ = _run(embed, labels, trace=False)
    return out


# revision 15
# speedup vs baseline: 2.5898x; 2.5898x over previous
"""Trainium2 Bass kernel for nn_CLloss (contrastive loss, anchor row 0).

Math (faithful to the torch/jax reference):
    e_j = x_j / max(||x_j||, 1e-12)          (row-normalize embed)
    d_j = ||(e_0 + 1e-6) - e_j||_2           (pairwise distance to anchor, j>=1)
    log_sim_j = -d_j / 0.1
    c_j = <labels_j, labels_0>
    Ci = 1e-12 + sum c_j ; Ei = 1e-12 + sum exp(log_sim_j)
    Li = sum -(c_j/Ci) * (log_sim_j - log Ei) ; loss = Li / n

With a = e_0 + 1e-6:  d_j^2 = ||a||^2 + 1 - 2*(a . x_j)/||x_j||, so the only
O(n*d) work is two per-row contractions over the feature dim: a.x_j and
sum_k x_jk^2.  Rows are sharded across 8 cores; each core gets its shard
TRANSPOSED (feature k on SBUF partitions, done on host) so the tensor engine
contracts over partitions:
  - a.x     via matmul(lhsT=[a_c | 0], rhs=x_tile)
  - sum x^2 via matmul(lhsT=[0 | 1],  rhs=square(x_tile))
Both accumulate into the SAME psum tile (row 0 = a.x, row 1 = sum x^2)
across the 16 feature chunks.  Squares are split between the scalar and
vector engines to stay under the DMA roofline.

Inputs are cast to bf16 on the host.  The loss is a mean over 16k rows, so
independent per-row rounding noise (~1e-4 in each d_j) averages down by
~sqrt(16384); measured end-to-end error vs the f32 reference is ~1e-5.
Device returns per-row (a.x, sum x^2); host does the O(n) epilogue in f64.
"""

import ml_dtypes
import numpy as np

import concourse.bacc as bacc
import concourse.bass as bass
import concourse.tile as tile
from concourse import mybir
from concourse.bass_utils import run_bass_kernel_spmd

N_ROWS = 16384
DIM = 2048
N_CORES = 8
ROWS_PER_CORE = N_ROWS // N_CORES  # 2048
KC = DIM // 128  # 16 feature chunks of 128 partitions
KP = KC // 2  # 8 chunk-pairs (1 MB DMA each)
JC = ROWS_PER_CORE // 512  # 4 row chunks of 512 (psum bank = 512 f32)

PD_EPS = 1e-6
NORM_EPS = 1e-12
T = 0.1

BF16 = ml_dtypes.bfloat16

_NC_CACHE = {}


def _build_bass():
    # Bacc (not raw Bass): its compile() legalizes sync waits — walrus accepts
    # at most ONE wait per instruction, and Tile freely emits several.
    nc = bacc.Bacc()
    f32 = mybir.dt.float32
    bf16 = mybir.dt.bfloat16
    xt = nc.dram_tensor("xt", [DIM, ROWS_PER_CORE], bf16, kind="ExternalInput")
    # Per feature chunk c, 4 weight columns: [a_c, 0, 0, 1].  The x-matmul
    # uses cols (a_c, 0) and the x^2-matmul cols (0, 1), so both accumulate
    # into the SAME psum tile: row 0 collects a.x only, row 1 sum x^2 only.
    aw = nc.dram_tensor("aw", [128, 4 * KC], bf16, kind="ExternalInput")
    out = nc.dram_tensor("out", [2, ROWS_PER_CORE], f32, kind="ExternalOutput")

    # view as chunk-pairs: pair p, partition q, free [b, j] with b in {0,1}
    xt_pairs = xt.rearrange("(p b q) j -> p q b j", b=2, q=128)

    with tile.TileContext(nc) as tc:
        with (
            tc.tile_pool(name="xp", bufs=4) as xp,
            tc.tile_pool(name="sqp", bufs=3) as sqp,
            tc.tile_pool(name="singles", bufs=1) as singles,
            tc.tile_pool(name="psum", bufs=1, space="PSUM") as psum,
        ):
            aw_sb = singles.tile([128, 4 * KC], bf16)
            nc.sync.dma_start(out=aw_sb[:], in_=aw[:])

            # Warmup matmul reading only aw_sb: lets PE observe the aw DMA
            # semaphore early so real matmuls need fewer waits.
            ps_warm = psum.tile([2, 2], f32, tag="pswarm", name="pswarm")
            nc.tensor.matmul(
                ps_warm[:], aw_sb[:, 0:2], aw_sb[:, 0:2], start=True, stop=True
            )

            ps = [
                psum.tile([2, 512], f32, tag=f"ps{j}", name=f"ps{j}")
                for j in range(JC)
            ]

            for p in range(KP):
                x_tile = xp.tile([128, 2, ROWS_PER_CORE], bf16)
                nc.sync.dma_start(out=x_tile[:], in_=xt_pairs[p])
                sq_tile = sqp.tile([128, 2, ROWS_PER_CORE], bf16)
                # squares: scalar engine does the first chunk of the pair,
                # vector engine (bf16 2x mode) the second.
                nc.scalar.activation(
                    out=sq_tile[:, 0, :],
                    in_=x_tile[:, 0, :],
                    func=mybir.ActivationFunctionType.Square,
                )
                nc.vector.tensor_mul(
                    sq_tile[:, 1, :], x_tile[:, 1, :], x_tile[:, 1, :]
                )
                for b in range(2):
                    c = 2 * p + b
                    w_x = aw_sb[:, 4 * c : 4 * c + 2]  # [a_c | 0]
                    w_q = aw_sb[:, 4 * c + 2 : 4 * c + 4]  # [0 | 1]
                    for j in range(JC):
                        nc.tensor.matmul(
                            ps[j][:],
                            w_x,
                            x_tile[:, b, j * 512 : (j + 1) * 512],
                            start=(c == 0),
                            stop=False,
                        )
                    for j in range(JC):
                        nc.tensor.matmul(
                            ps[j][:],
                            w_q,
                            sq_tile[:, b, j * 512 : (j + 1) * 512],
                            start=False,
                            stop=(c == KC - 1),
                        )

            out_sb = singles.tile([2, ROWS_PER_CORE], f32)
            for j in range(JC):
                nc.vector.tensor_copy(
                    out_sb[0:2, j * 512 : (j + 1) * 512], ps[j][:]
                )
            nc.sync.dma_start(out=out[:], in_=out_sb[:])

    nc.compile()
    return nc


def _get_nc():
    if "nc" not in _NC_CACHE:
        _NC_CACHE["nc"] = _build_bass()
    return _NC_CACHE["nc"]


def _make_in_maps(embed):
    x0 = embed[0].astype(np.float64)
    nrm0 = max(np.sqrt(np.dot(x0, x0)), NORM_EPS)
    a64 = x0 / nrm0 + PD_EPS

    aw = np.zeros((128, 4 * KC), BF16)
    a16 = a64.astype(BF16)
    for c in range(KC):
        aw[:, 4 * c] = a16[c * 128 : (c + 1) * 128]
        aw[:, 4 * c + 3] = 1.0

    in_maps = []
    for core in range(N_CORES):
        shard = embed[core * ROWS_PER_CORE : (core + 1) * ROWS_PER_CORE]
        xt = shard.T.astype(BF16)  # [DIM, ROWS_PER_CORE], C-contiguous
        in_maps.append({"xt": xt, "aw": aw})
    return in_maps, a64


def _epilogue(results, a64, labels):
    adot = np.concatenate([r["out"][0] for r in results]).astype(np.float64)
    ss = np.concatenate([r["out"][1] for r in results]).astype(np.float64)

    nrm = np.maximum(np.sqrt(ss), NORM_EPS)
    t = adot / nrm  # a . e_j
    a2 = np.dot(a64, a64)
    d2 = np.maximum(a2 + 1.0 - 2.0 * t, 0.0)
    d = np.sqrt(d2)[1:]  # anchor row excluded, j = 1..n-1

    lab = labels.astype(np.float64)
    c = lab[1:] @ lab[0]
    ci = 1e-12 + c.sum()
    log_sim = -d / T
    ei = 1e-12 + np.exp(log_sim).sum()
    li = (-(c / ci) * (log_sim - np.log(ei))).sum()
    return np.asarray(li / N_ROWS, dtype=np.float32)


def _run(embed, labels, trace=False):
    embed = np.ascontiguousarray(np.asarray(embed, dtype=np.float32))
    labels = np.asarray(labels)
    assert embed.shape == (N_ROWS, DIM), embed.shape

    nc = _get_nc()
    in_maps, a64 = _make_in_maps(embed)
    kwargs = {"trace_cores": list(range(N_CORES))} if trace else {}
    res = run_bass_kernel_spmd(
        nc, in_maps, core_ids=list(range(N_CORES)), trace=trace, **kwargs
    )
    return _epilogue(res.results, a64, labels), res


def kernel(embed, labels):
    out, _ = _run(embed, labels, trace=False)
    return out


# revision 18
# speedup vs baseline: 2.6429x; 1.0205x over previous
"""Trainium2 Bass kernel for nn_CLloss (contrastive loss, anchor row 0).

Math (faithful to the torch/jax reference):
    e_j = x_j / max(||x_j||, 1e-12)          (row-normalize embed)
    d_j = ||(e_0 + 1e-6) - e_j||_2           (pairwise distance to anchor, j>=1)
    log_sim_j = -d_j / 0.1
    c_j = <labels_j, labels_0>
    Ci = 1e-12 + sum c_j ; Ei = 1e-12 + sum exp(log_sim_j)
    Li = sum -(c_j/Ci) * (log_sim_j - log Ei) ; loss = Li / n

With a = e_0 + 1e-6:  d_j^2 = ||a||^2 + 1 - 2*(a . x_j)/||x_j||, so the only
O(n*d) work is two per-row contractions over the feature dim: a.x_j and
sum_k x_jk^2.  Rows are sharded across 8 cores; each core gets its shard
TRANSPOSED (feature k on SBUF partitions, done on host) so the tensor engine
contracts over partitions:
  - a.x     via matmul(lhsT=[a_c | 0], rhs=x_tile)
  - sum x^2 via matmul(lhsT=[0 | 1],  rhs=square(x_tile))
Both accumulate into the SAME psum tile (row 0 = a.x, row 1 = sum x^2)
across the 16 feature chunks.  Squares are split between the scalar and
vector engines to stay under the DMA roofline.

Inputs are cast to bf16 on the host.  The loss is a mean over 16k rows, so
independent per-row rounding noise (~1e-4 in each d_j) averages down by
~sqrt(16384); measured end-to-end error vs the f32 reference is ~1e-5.
Device returns per-row (a.x, sum x^2); host does the O(n) epilogue in f64.
"""

import ml_dtypes
import numpy as np

import concourse.bacc as bacc
import concourse.bass as bass
import concourse.tile as tile
from concourse import mybir
from concourse.bass_utils import run_bass_kernel_spmd
from concourse.tile import add_dep_helper

N_ROWS = 16384
DIM = 2048
N_CORES = 8
ROWS_PER_CORE = N_ROWS // N_CORES  # 2048
KC = DIM // 128  # 16 feature chunks of 128 partitions
KP = KC // 2  # 8 chunk-pairs (1 MB DMA each)
JC = ROWS_PER_CORE // 512  # 4 row chunks of 512 (psum bank = 512 f32)

PD_EPS = 1e-6
NORM_EPS = 1e-12
T = 0.1

BF16 = ml_dtypes.bfloat16

_NC_CACHE = {}


def _build_bass():
    # Bacc (not raw Bass): its compile() legalizes sync waits — walrus accepts
    # at most ONE wait per instruction, and Tile freely emits several.
    nc = bacc.Bacc()
    f32 = mybir.dt.float32
    bf16 = mybir.dt.bfloat16
    xt = nc.dram_tensor("xt", [DIM, ROWS_PER_CORE], bf16, kind="ExternalInput")
    # Per feature chunk c, 4 weight columns: [a_c, 0, 0, 1].  The x-matmul
    # uses cols (a_c, 0) and the x^2-matmul cols (0, 1), so both accumulate
    # into the SAME psum tile: row 0 collects a.x only, row 1 sum x^2 only.
    aw = nc.dram_tensor("aw", [128, 4 * KC], bf16, kind="ExternalInput")
    out = nc.dram_tensor("out", [2, ROWS_PER_CORE], f32, kind="ExternalOutput")

    # view as chunk-pairs: pair p, partition q, free [b, j] with b in {0,1}
    xt_pairs = xt.rearrange("(p b q) j -> p q b j", b=2, q=128)

    with tile.TileContext(nc) as tc:
        with (
            tc.tile_pool(name="xp", bufs=4) as xp,
            tc.tile_pool(name="sqp", bufs=3) as sqp,
            tc.tile_pool(name="singles", bufs=1) as singles,
            tc.tile_pool(name="psum", bufs=1, space="PSUM") as psum,
        ):
            aw_sb = singles.tile([128, 4 * KC], bf16)
            nc.sync.dma_start(out=aw_sb[:], in_=aw[:])

            ps = [
                psum.tile([2, 512], f32, tag=f"ps{j}", name=f"ps{j}")
                for j in range(JC)
            ]

            # All matmuls are chained in program order on PE (order-only
            # deps, no semaphores).  That makes it safe to skip the
            # per-matmul LDWEIGHTS on the 3 trailing matmuls of each
            # same-weights group of 4 — the redundant weight reloads
            # otherwise serialize against the previous matmul and nearly
            # double PE time.
            prev_mm = None

            def mm(out_ap, w, rhs, start, stop, reuse_w):
                nonlocal prev_mm
                inst = nc.tensor.matmul(
                    out_ap, w, rhs, start=start, stop=stop
                ).ins
                if reuse_w:
                    inst.ldweights = False
                if prev_mm is not None:
                    add_dep_helper(inst, prev_mm, reason="pe program order")
                prev_mm = inst

            for p in range(KP):
                x_tile = xp.tile([128, 2, ROWS_PER_CORE], bf16)
                nc.sync.dma_start(out=x_tile[:], in_=xt_pairs[p])
                sq_tile = sqp.tile([128, 2, ROWS_PER_CORE], bf16)
                # squares: scalar engine does the first chunk of the pair,
                # vector engine (bf16 2x mode) the second.
                nc.scalar.activation(
                    out=sq_tile[:, 0, :],
                    in_=x_tile[:, 0, :],
                    func=mybir.ActivationFunctionType.Square,
                )
                nc.vector.tensor_mul(
                    sq_tile[:, 1, :], x_tile[:, 1, :], x_tile[:, 1, :]
                )
                for b in range(2):
                    c = 2 * p + b
                    w_x = aw_sb[:, 4 * c : 4 * c + 2]  # [a_c | 0]
                    w_q = aw_sb[:, 4 * c + 2 : 4 * c + 4]  # [0 | 1]
                    for j in range(JC):
                        mm(
                            ps[j][:],
                            w_x,
                            x_tile[:, b, j * 512 : (j + 1) * 512],
                            start=(c == 0),
                            stop=False,
                            reuse_w=(j > 0),
                        )
                    for j in range(JC):
                        mm(
                            ps[j][:],
                            w_q,
                            sq_tile[:, b, j * 512 : (j + 1) * 512],
                            start=False,
                            stop=(c == KC - 1),
                            reuse_w=(j > 0),
                        )

            out_sb = singles.tile([2, ROWS_PER_CORE], f32)
            for j in range(JC):
                dst = out_sb[0:2, j * 512 : (j + 1) * 512]
                if j % 2 == 0:
                    nc.vector.tensor_copy(dst, ps[j][:])
                else:
                    nc.scalar.copy(dst, ps[j][:])
            nc.sync.dma_start(out=out[:], in_=out_sb[:])

    nc.compile()
    return nc


def _get_nc():
    if "nc" not in _NC_CACHE:
        _NC_CACHE["nc"] = _build_bass()
    return _NC_CACHE["nc"]


def _make_in_maps(embed):
    x0 = embed[0].astype(np.float64)
    nrm0 = max(np.sqrt(np.dot(x0, x0)), NORM_EPS)
    a64 = x0 / nrm0 + PD_EPS

    aw = np.zeros((128, 4 * KC), BF16)
    a16 = a64.astype(BF16)
    for c in range(KC):
        aw[:, 4 * c] = a16[c * 128 : (c + 1) * 128]
        aw[:, 4 * c + 3] = 1.0

    in_maps = []
    for core in range(N_CORES):
        shard = embed[core * ROWS_PER_CORE : (core + 1) * ROWS_PER_CORE]
        xt = shard.T.astype(BF16)  # [DIM, ROWS_PER_CORE], C-contiguous
        in_maps.append({"xt": xt, "aw": aw})
    return in_maps, a64


def _epilogue(results, a64, labels):
    adot = np.concatenate([r["out"][0] for r in results]).astype(np.float64)
    ss = np.concatenate([r["out"][1] for r in results]).astype(np.float64)

    nrm = np.maximum(np.sqrt(ss), NORM_EPS)
    t = adot / nrm  # a . e_j
    a2 = np.dot(a64, a64)
    d2 = np.maximum(a2 + 1.0 - 2.0 * t, 0.0)
    d = np.sqrt(d2)[1:]  # anchor row excluded, j = 1..n-1

    lab = labels.astype(np.float64)
    c = lab[1:] @ lab[0]
    ci = 1e-12 + c.sum()
    log_sim = -d / T
    ei = 1e-12 + np.exp(log_sim).sum()
    li = (-(c / ci) * (log_sim - np.log(ei))).sum()
    return np.asarray(li / N_ROWS, dtype=np.float32)


def _run(embed, labels, trace=False):
    embed = np.ascontiguousarray(np.asarray(embed, dtype=np.float32))
    labels = np.asarray(labels)
    assert embed.shape == (N_ROWS, DIM), embed.shape

    nc = _get_nc()
    in_maps, a64 = _make_in_maps(embed)
    kwargs = {"trace_cores": list(range(N_CORES))} if trace else {}
    res = run_bass_kernel_spmd(
        nc, in_maps, core_ids=list(range(N_CORES)), trace=trace, **kwargs
    )
    return _epilogue(res.results, a64, labels), res


def kernel(embed, labels):
    out, _ = _run(embed, labels, trace=False)
    return out
